# revision 1
# baseline (speedup 1.0000x reference)
"""Trainium2 Bass kernel for nn_NaiveCustomLSTM (8 NeuronCores, SPMD).

The reference module's masks make this 20 independent 32-wide LSTMs
(V_MASK = kron(eye(20), ones(32,32)); U_MASK selects 1-2 inputs per
feature).  Strategy: shard the 20 feature-LSTMs across 8 cores
({3,3,3,3,2,2,2,2} with zero-padded third slots), full batch B=128 per
core, K-packed into one 96-row block per core.

Per core / per step t:
  gates = [Wu; bias]^T [xg_t; 1] (+) Wv^T h_{t-1}   via PSUM accumulation
  (U-matmuls for t+1 are issued off the h-critical-path to keep PE warm)
  sigma(i,f) -> TT mul [P|Q] -> TT add -> c (PSUM-resident, fp32)
  tanh(c) -> h = sigma(o) * tanh(c)  (h stored fp16, feeds next V-matmuls)
Matmul operands are fp16 (fp32 matmul costs 4 cycles/row on TRN2 PE;
fp16 costs 1); the c accumulation path stays fp32.  End-to-end rel err
vs the fp32 reference ~6e-4.
"""
import sys, os, types

sys.path.insert(0, "/opt/trn_rl_repo")

import numpy as np

# ---- problem constants (hardcoded from the reference module) ----
INPUT_SZ, HPF = 16, 32
INTERACTIONS = [(0, 1), (2, 3), (4, 5), (6, 7)]
N_INT = len(INTERACTIONS)
INPUT_SIZE = INPUT_SZ + 2 * N_INT            # 24
NFEAT = INPUT_SZ + N_INT                     # 20 feature blocks
HIDDEN = NFEAT * HPF                         # 640
BATCH, SEQ = 128, 512
FEAT_SEQ = np.array(list(range(INPUT_SZ)) + [f for p in INTERACTIONS for f in p])

F = 3            # feature slots per core
HR = 32 * F      # 96 h rows per core
UR = 25          # 24 gathered inputs + ones row
B = BATCH
CH = 16          # xg prefetch chunk (steps)

_Um = np.zeros((INPUT_SIZE, HIDDEN), np.float32)
for _i in range(INPUT_SZ):
    _Um[_i, _i * HPF:(_i + 1) * HPF] = 1
for _i in range(0, N_INT, 2):
    _Um[_i + INPUT_SZ, _i * HPF:(_i + 1) * HPF] = 1
    _Um[_i + INPUT_SZ + 1, _i * HPF:(_i + 1) * HPF] = 1

CORE_FEATS = [list(range(3 * c, 3 * c + 3)) for c in range(4)] + \
             [list(range(12 + 2 * (c - 4), 14 + 2 * (c - 4))) for c in range(4, 8)]

GATE_ORDER = ["i", "f", "o", "c"]   # reference gate names; bank order i, f, o, g~

_NC_CACHE = {}


def _ensure_ntff_hook():
    """Register the axon NTFF profile hook if the antenv stub lacks it
    (harmless if tracing is never requested)."""
    try:
        import antenv
        if not hasattr(antenv, "axon_hooks"):
            mod = types.ModuleType("antenv.axon_hooks")
            _HOOK = [None]
            mod.set_axon_ntff_profile_hook = lambda h: _HOOK.__setitem__(0, h)
            mod.get_axon_ntff_profile_hook = lambda: _HOOK[0]
            sys.modules["antenv.axon_hooks"] = mod
            antenv.axon_hooks = mod
            try:
                from trn_agent_boot.trn_boot import _ntff_profile_via_ctypes
                mod.set_axon_ntff_profile_hook(
                    _ntff_profile_via_ctypes("/opt/axon/libaxon_pjrt.so"))
            except Exception:
                pass
    except Exception:
        pass


def _build(T):
    import concourse.bass as bass
    import concourse.tile as tile
    from concourse import bacc, mybir

    AF = mybir.ActivationFunctionType
    ALU = mybir.AluOpType
    FP32 = mybir.dt.float32
    FP16 = mybir.dt.float16

    nc = bacc.Bacc("TRN2", target_bir_lowering=False, debug=False)
    Wv_d = nc.dram_tensor("Wv", [HR, 4, HR], FP16, kind="ExternalInput")
    Wu_d = nc.dram_tensor("Wu", [UR, 4, HR], FP16, kind="ExternalInput")
    xg_d = nc.dram_tensor("xg", [UR, T + CH, B], FP16, kind="ExternalInput")
    hseq_d = nc.dram_tensor("hseq", [T, HR, B], FP16, kind="ExternalOutput")
    cout_d = nc.dram_tensor("cout", [HR, B], FP32, kind="ExternalOutput")

    with tile.TileContext(nc) as tc:
        with (
            tc.tile_pool(name="consts", bufs=1) as consts,
            tc.tile_pool(name="hpool", bufs=4) as hpool,
            tc.tile_pool(name="xgp", bufs=2) as xgp,
            tc.tile_pool(name="psum", bufs=2, space="PSUM") as psum_pool,
            tc.tile_pool(name="work", bufs=3) as work,
        ):
            Wv = consts.tile([HR, 4, HR], FP16)
            nc.sync.dma_start(Wv[:], Wv_d.ap())
            Wu = consts.tile([UR, 4, HR], FP16)
            nc.sync.dma_start(Wu[:], Wu_d.ap())
            # TgC (PSUM-resident): slot 0 = tanh(g~_t), slot 1 = c (fp32)
            TgC = psum_pool.tile([HR, 2, 256], FP32, tag="tgc")
            nc.vector.memset(TgC[:, 1, 0:B], 0.0)

            h = hpool.tile([HR, B], FP16, tag="h")
            nc.vector.memset(h[:], 0.0)

            xgc = xgp.tile([UR, CH, B], FP16, tag="xg")
            nc.sync.dma_start(xgc[:], xg_d.ap()[:, 0:CH, :])

            # U-matmuls for t=0.  ps_if = gates (i, f); ps_go = (g~, o).
            ps_if = psum_pool.tile([HR, 2, 256], FP32, tag="ps_if")
            ps_go = psum_pool.tile([HR, 2, 256], FP32, tag="ps_go")
            for sl, g in ((0, 0), (1, 1)):
                nc.tensor.matmul(ps_if[:, sl, 0:B], Wu[:, g, :], xgc[:, 0, :],
                                 start=(sl == 0), stop=False)
            for sl, g in ((0, 3), (1, 2)):
                nc.tensor.matmul(ps_go[:, sl, 0:B], Wu[:, g, :], xgc[:, 0, :],
                                 start=(sl == 0), stop=False)

            for t in range(T):
                j = t % CH
                if j == 0 and t + CH < T + CH:
                    xgc_next = xgp.tile([UR, CH, B], FP16, tag="xg")
                    nc.sync.dma_start(xgc_next[:],
                                      xg_d.ap()[:, t + CH:t + 2 * CH, :])
                # V-matmuls accumulate onto xu; (i, f) first -> sigma starts early
                for sl, g in ((0, 0), (1, 1)):
                    nc.tensor.matmul(ps_if[:, sl, 0:B], Wv[:, g, :], h[:],
                                     start=False, stop=True)
                for sl, g in ((0, 3), (1, 2)):
                    nc.tensor.matmul(ps_go[:, sl, 0:B], Wv[:, g, :], h[:],
                                     start=False, stop=True)
                SIF = work.tile([HR, 2, B], FP32, tag="sif")
                nc.scalar.activation(SIF[:], ps_if[:, :, 0:B], AF.Sigmoid)
                nc.scalar.activation(TgC[:, 0, 0:B], ps_go[:, 0, 0:B], AF.Tanh)
                So = work.tile([HR, B], FP32, tag="so")
                nc.scalar.activation(So[:], ps_go[:, 1, 0:B], AF.Sigmoid)

                # U-matmuls for t+1 (off the h chain; keeps PE warm)
                ps_if_n = psum_pool.tile([HR, 2, 256], FP32, tag="ps_if")
                ps_go_n = psum_pool.tile([HR, 2, 256], FP32, tag="ps_go")
                xgn, jn = (xgc, j + 1) if j + 1 < CH else (xgc_next, 0)
                for sl, g in ((0, 0), (1, 1)):
                    nc.tensor.matmul(ps_if_n[:, sl, 0:B], Wu[:, g, :],
                                     xgn[:, jn, :], start=(sl == 0), stop=False)
                for sl, g in ((0, 3), (1, 2)):
                    nc.tensor.matmul(ps_go_n[:, sl, 0:B], Wu[:, g, :],
                                     xgn[:, jn, :], start=(sl == 0), stop=False)

                PQ = work.tile([HR, 2, B], FP32, tag="pq")
                nc.vector.tensor_tensor(PQ[:], SIF[:], TgC[:, :, 0:B], op=ALU.mult)
                nc.vector.tensor_tensor(TgC[:, 1, 0:B], PQ[:, 0, :], PQ[:, 1, :],
                                        op=ALU.add)
                Tc = work.tile([HR, B], FP32, tag="tc")
                nc.scalar.activation(Tc[:], TgC[:, 1, 0:B], AF.Tanh)

                h = hpool.tile([HR, B], FP16, tag="h")
                nc.vector.tensor_tensor(h[:], So[:], Tc[:], op=ALU.mult)
                nc.gpsimd.dma_start(hseq_d[t], h[:])
                if j == CH - 1:
                    xgc = xgc_next
                ps_if, ps_go = ps_if_n, ps_go_n

            c_sb = work.tile([HR, B], FP32, tag="csb")
            nc.vector.tensor_copy(c_sb[:], TgC[:, 1, 0:B])
            nc.sync.dma_start(cout_d.ap(), c_sb[:])
    nc.compile()
    return nc


def _make_W(core, weights):
    Wv = np.zeros((HR, 4, HR), np.float32)
    Wu = np.zeros((UR, 4, HR), np.float32)
    for gs, name in enumerate(GATE_ORDER):
        U = weights[f"U_{name}"] * _Um
        V = weights[f"V_{name}"]
        b = weights[f"b_{name}"]
        for sl, f in enumerate(CORE_FEATS[core]):
            r0, c0 = 32 * sl, 32 * f
            Wv[r0:r0 + 32, gs, r0:r0 + 32] = V[c0:c0 + 32, c0:c0 + 32]
            Wu[:INPUT_SIZE, gs, r0:r0 + 32] = U[:, c0:c0 + 32]
            Wu[INPUT_SIZE, gs, r0:r0 + 32] = b[c0:c0 + 32]
    return Wv.astype(np.float16), Wu.astype(np.float16)


def kernel(x, U_i, V_i, b_i, U_f, V_f, b_f, U_c, V_c, b_c, U_o, V_o, b_o,
           _trace=False):
    _ensure_ntff_hook()
    from concourse.bass_utils import run_bass_kernel_spmd

    x = np.asarray(x)
    weights = {"U_i": np.asarray(U_i), "V_i": np.asarray(V_i), "b_i": np.asarray(b_i),
               "U_f": np.asarray(U_f), "V_f": np.asarray(V_f), "b_f": np.asarray(b_f),
               "U_c": np.asarray(U_c), "V_c": np.asarray(V_c), "b_c": np.asarray(b_c),
               "U_o": np.asarray(U_o), "V_o": np.asarray(V_o), "b_o": np.asarray(b_o)}
    T = x.shape[1]

    if T not in _NC_CACHE:
        _NC_CACHE[T] = _build(T)
    nc = _NC_CACHE[T]

    # xg [25, T+CH, B] fp16 with ones row
    xg = np.zeros((UR, T + CH, B), np.float32)
    xg[:INPUT_SIZE, :T] = x[:, :, FEAT_SEQ].transpose(2, 1, 0)
    xg[INPUT_SIZE, :T] = 1.0
    xg = xg.astype(np.float16)

    in_maps = []
    for c in range(8):
        Wv, Wu = _make_W(c, weights)
        in_maps.append({"Wv": Wv, "Wu": Wu, "xg": xg})

    res = run_bass_kernel_spmd(nc, in_maps, core_ids=list(range(8)), trace=_trace)

    H = np.zeros((B, T, HIDDEN), np.float32)
    cT = np.zeros((B, HIDDEN), np.float32)
    for core in range(8):
        hs = res.results[core]["hseq"].astype(np.float32).transpose(2, 0, 1)
        co = res.results[core]["cout"].transpose(1, 0)
        for sl, f in enumerate(CORE_FEATS[core]):
            H[:, :, 32 * f:32 * f + 32] = hs[:, :, 32 * sl:32 * sl + 32]
            cT[:, 32 * f:32 * f + 32] = co[:, 32 * sl:32 * sl + 32]
    hT = H[:, -1, :].copy()
    if _trace:
        kernel.last_exec_time_ns = res.exec_time_ns
        kernel.last_trace = (res.instructions_and_trace or (None, None))[1]
    return H, (hT, cT)


# revision 2
# speedup vs baseline: 1.0127x; 1.0127x over previous
"""Trainium2 Bass kernel for nn_NaiveCustomLSTM (8 NeuronCores, SPMD).

The reference module's masks make this 20 independent 32-wide LSTMs
(V_MASK = kron(eye(20), ones(32,32)); U_MASK selects 1-2 inputs per
feature).  Strategy: shard the 20 feature-LSTMs across 8 cores
({3,3,3,3,2,2,2,2} with zero-padded third slots), full batch B=128 per
core, K-packed into one 96-row block per core.

Per core / per step t:
  gates = [Wu; bias]^T [xg_t; 1] (+) Wv^T h_{t-1}   via PSUM accumulation
  (U-matmuls for t+1 are issued off the h-critical-path to keep PE warm)
  sigma(i,f) -> TT mul [P|Q] -> TT add -> c (PSUM-resident, fp32)
  tanh(c) -> h = sigma(o) * tanh(c)  (h stored fp16, feeds next V-matmuls)
Matmul operands are fp16 (fp32 matmul costs 4 cycles/row on TRN2 PE;
fp16 costs 1); the c accumulation path stays fp32.  End-to-end rel err
vs the fp32 reference ~6e-4.
"""
import sys, os, types

sys.path.insert(0, "/opt/trn_rl_repo")

import numpy as np

# ---- problem constants (hardcoded from the reference module) ----
INPUT_SZ, HPF = 16, 32
INTERACTIONS = [(0, 1), (2, 3), (4, 5), (6, 7)]
N_INT = len(INTERACTIONS)
INPUT_SIZE = INPUT_SZ + 2 * N_INT            # 24
NFEAT = INPUT_SZ + N_INT                     # 20 feature blocks
HIDDEN = NFEAT * HPF                         # 640
BATCH, SEQ = 128, 512
FEAT_SEQ = np.array(list(range(INPUT_SZ)) + [f for p in INTERACTIONS for f in p])

F = 3            # feature slots per core
HR = 32 * F      # 96 h rows per core
UR = 25          # 24 gathered inputs + ones row
B = BATCH
CH = 16          # xg prefetch chunk (steps)

_Um = np.zeros((INPUT_SIZE, HIDDEN), np.float32)
for _i in range(INPUT_SZ):
    _Um[_i, _i * HPF:(_i + 1) * HPF] = 1
for _i in range(0, N_INT, 2):
    _Um[_i + INPUT_SZ, _i * HPF:(_i + 1) * HPF] = 1
    _Um[_i + INPUT_SZ + 1, _i * HPF:(_i + 1) * HPF] = 1

CORE_FEATS = [list(range(3 * c, 3 * c + 3)) for c in range(4)] + \
             [list(range(12 + 2 * (c - 4), 14 + 2 * (c - 4))) for c in range(4, 8)]

GATE_ORDER = ["i", "f", "o", "c"]   # reference gate names; bank order i, f, o, g~

_NC_CACHE = {}


def _ensure_ntff_hook():
    """Register the axon NTFF profile hook if the antenv stub lacks it
    (harmless if tracing is never requested)."""
    try:
        import antenv
        if not hasattr(antenv, "axon_hooks"):
            mod = types.ModuleType("antenv.axon_hooks")
            _HOOK = [None]
            mod.set_axon_ntff_profile_hook = lambda h: _HOOK.__setitem__(0, h)
            mod.get_axon_ntff_profile_hook = lambda: _HOOK[0]
            sys.modules["antenv.axon_hooks"] = mod
            antenv.axon_hooks = mod
            try:
                from trn_agent_boot.trn_boot import _ntff_profile_via_ctypes
                mod.set_axon_ntff_profile_hook(
                    _ntff_profile_via_ctypes("/opt/axon/libaxon_pjrt.so"))
            except Exception:
                pass
    except Exception:
        pass


def _build(T):
    import concourse.bass as bass
    import concourse.tile as tile
    from concourse import bacc, mybir

    AF = mybir.ActivationFunctionType
    ALU = mybir.AluOpType
    FP32 = mybir.dt.float32
    FP16 = mybir.dt.float16

    nc = bacc.Bacc("TRN2", target_bir_lowering=False, debug=False)
    Wv_d = nc.dram_tensor("Wv", [HR, 4, HR], FP16, kind="ExternalInput")
    Wu_d = nc.dram_tensor("Wu", [UR, 4, HR], FP16, kind="ExternalInput")
    xg_d = nc.dram_tensor("xg", [UR, T + CH, B], FP16, kind="ExternalInput")
    hseq_d = nc.dram_tensor("hseq", [T, HR, B], FP16, kind="ExternalOutput")
    cout_d = nc.dram_tensor("cout", [HR, B], FP32, kind="ExternalOutput")

    with tile.TileContext(nc) as tc:
        with (
            tc.tile_pool(name="consts", bufs=1) as consts,
            tc.tile_pool(name="hpool", bufs=4) as hpool,
            tc.tile_pool(name="xgp", bufs=2) as xgp,
            tc.tile_pool(name="psum", bufs=2, space="PSUM") as psum_pool,
            tc.tile_pool(name="psumc", bufs=1, space="PSUM") as psumc_pool,
            tc.tile_pool(name="work", bufs=3) as work,
        ):
            Wv = consts.tile([HR, 4, HR], FP16)
            nc.sync.dma_start(Wv[:], Wv_d.ap())
            Wu = consts.tile([UR, 4, HR], FP16)
            nc.sync.dma_start(Wu[:], Wu_d.ap())
            # TgC (PSUM-resident): slot 0 = tanh(g~_t), slot 1 = c (fp32)
            TgC = psumc_pool.tile([HR, 2, 256], FP32, tag="tgc")
            nc.vector.memset(TgC[:, 1, 0:B], 0.0)

            h = hpool.tile([HR, B], FP16, tag="h")
            nc.vector.memset(h[:], 0.0)

            xgc = xgp.tile([UR, CH, B], FP16, tag="xg")
            nc.sync.dma_start(xgc[:], xg_d.ap()[:, 0:CH, :])

            # U-matmuls for t=0.  ps_if = gates (i, f); ps_go = (g~, o).
            ps_if = psum_pool.tile([HR, 2, 256], FP32, tag="ps_if")
            ps_go = psum_pool.tile([HR, 2, 256], FP32, tag="ps_go")
            for sl, g in ((0, 0), (1, 1)):
                nc.tensor.matmul(ps_if[:, sl, 0:B], Wu[:, g, :], xgc[:, 0, :],
                                 start=(sl == 0), stop=False)
            for sl, g in ((0, 3), (1, 2)):
                nc.tensor.matmul(ps_go[:, sl, 0:B], Wu[:, g, :], xgc[:, 0, :],
                                 start=(sl == 0), stop=False)

            for t in range(T):
                j = t % CH
                if j == 0 and t + CH < T + CH:
                    xgc_next = xgp.tile([UR, CH, B], FP16, tag="xg")
                    nc.sync.dma_start(xgc_next[:],
                                      xg_d.ap()[:, t + CH:t + 2 * CH, :])
                # V-matmuls accumulate onto xu; (i, f) first -> sigma starts early
                for sl, g in ((0, 0), (1, 1)):
                    nc.tensor.matmul(ps_if[:, sl, 0:B], Wv[:, g, :], h[:],
                                     start=False, stop=True)
                for sl, g in ((0, 3), (1, 2)):
                    nc.tensor.matmul(ps_go[:, sl, 0:B], Wv[:, g, :], h[:],
                                     start=False, stop=True)
                SIF = work.tile([HR, 2, B], FP32, tag="sif")
                nc.scalar.activation(SIF[:], ps_if[:, :, 0:B], AF.Sigmoid)
                nc.scalar.activation(TgC[:, 0, 0:B], ps_go[:, 0, 0:B], AF.Tanh)
                So = work.tile([HR, B], FP32, tag="so")
                nc.scalar.activation(So[:], ps_go[:, 1, 0:B], AF.Sigmoid)

                # U-matmuls for t+1 (off the h chain; keeps PE warm)
                ps_if_n = psum_pool.tile([HR, 2, 256], FP32, tag="ps_if")
                ps_go_n = psum_pool.tile([HR, 2, 256], FP32, tag="ps_go")
                xgn, jn = (xgc, j + 1) if j + 1 < CH else (xgc_next, 0)
                for sl, g in ((0, 0), (1, 1)):
                    nc.tensor.matmul(ps_if_n[:, sl, 0:B], Wu[:, g, :],
                                     xgn[:, jn, :], start=(sl == 0), stop=False)
                for sl, g in ((0, 3), (1, 2)):
                    nc.tensor.matmul(ps_go_n[:, sl, 0:B], Wu[:, g, :],
                                     xgn[:, jn, :], start=(sl == 0), stop=False)

                PQ = work.tile([HR, 2, B], FP32, tag="pq")
                nc.vector.tensor_tensor(PQ[:], SIF[:], TgC[:, :, 0:B], op=ALU.mult)
                nc.vector.tensor_tensor(TgC[:, 1, 0:B], PQ[:, 0, :], PQ[:, 1, :],
                                        op=ALU.add)
                Tc = psumc_pool.tile([HR, 512], FP32, tag="tc")
                nc.scalar.activation(Tc[:, 0:B], TgC[:, 1, 0:B], AF.Tanh)

                h = hpool.tile([HR, B], FP16, tag="h")
                nc.vector.tensor_tensor(h[:], So[:], Tc[:, 0:B], op=ALU.mult)
                nc.gpsimd.dma_start(hseq_d[t], h[:])
                if j == CH - 1:
                    xgc = xgc_next
                ps_if, ps_go = ps_if_n, ps_go_n

            c_sb = work.tile([HR, B], FP32, tag="csb")
            nc.vector.tensor_copy(c_sb[:], TgC[:, 1, 0:B])
            nc.sync.dma_start(cout_d.ap(), c_sb[:])
    nc.compile()
    return nc


def _make_W(core, weights):
    Wv = np.zeros((HR, 4, HR), np.float32)
    Wu = np.zeros((UR, 4, HR), np.float32)
    for gs, name in enumerate(GATE_ORDER):
        U = weights[f"U_{name}"] * _Um
        V = weights[f"V_{name}"]
        b = weights[f"b_{name}"]
        for sl, f in enumerate(CORE_FEATS[core]):
            r0, c0 = 32 * sl, 32 * f
            Wv[r0:r0 + 32, gs, r0:r0 + 32] = V[c0:c0 + 32, c0:c0 + 32]
            Wu[:INPUT_SIZE, gs, r0:r0 + 32] = U[:, c0:c0 + 32]
            Wu[INPUT_SIZE, gs, r0:r0 + 32] = b[c0:c0 + 32]
    return Wv.astype(np.float16), Wu.astype(np.float16)


def kernel(x, U_i, V_i, b_i, U_f, V_f, b_f, U_c, V_c, b_c, U_o, V_o, b_o,
           _trace=False):
    _ensure_ntff_hook()
    from concourse.bass_utils import run_bass_kernel_spmd

    x = np.asarray(x)
    weights = {"U_i": np.asarray(U_i), "V_i": np.asarray(V_i), "b_i": np.asarray(b_i),
               "U_f": np.asarray(U_f), "V_f": np.asarray(V_f), "b_f": np.asarray(b_f),
               "U_c": np.asarray(U_c), "V_c": np.asarray(V_c), "b_c": np.asarray(b_c),
               "U_o": np.asarray(U_o), "V_o": np.asarray(V_o), "b_o": np.asarray(b_o)}
    T = x.shape[1]

    if T not in _NC_CACHE:
        _NC_CACHE[T] = _build(T)
    nc = _NC_CACHE[T]

    # xg [25, T+CH, B] fp16 with ones row
    xg = np.zeros((UR, T + CH, B), np.float32)
    xg[:INPUT_SIZE, :T] = x[:, :, FEAT_SEQ].transpose(2, 1, 0)
    xg[INPUT_SIZE, :T] = 1.0
    xg = xg.astype(np.float16)

    in_maps = []
    for c in range(8):
        Wv, Wu = _make_W(c, weights)
        in_maps.append({"Wv": Wv, "Wu": Wu, "xg": xg})

    res = run_bass_kernel_spmd(nc, in_maps, core_ids=list(range(8)), trace=_trace)

    H = np.zeros((B, T, HIDDEN), np.float32)
    cT = np.zeros((B, HIDDEN), np.float32)
    for core in range(8):
        hs = res.results[core]["hseq"].astype(np.float32).transpose(2, 0, 1)
        co = res.results[core]["cout"].transpose(1, 0)
        for sl, f in enumerate(CORE_FEATS[core]):
            H[:, :, 32 * f:32 * f + 32] = hs[:, :, 32 * sl:32 * sl + 32]
            cT[:, 32 * f:32 * f + 32] = co[:, 32 * sl:32 * sl + 32]
    hT = H[:, -1, :].copy()
    if _trace:
        kernel.last_exec_time_ns = res.exec_time_ns
        kernel.last_trace = (res.instructions_and_trace or (None, None))[1]
    return H, (hT, cT)
